# revision 11
# baseline (speedup 1.0000x reference)
import sys

sys.path.insert(0, "/opt/trn_rl_repo")
import numpy as np
from contextlib import ExitStack

from concourse import bacc
import concourse.tile as tile
from concourse import mybir
from concourse.bass_utils import run_bass_kernel_spmd

fp32 = mybir.dt.float32
fp32r = mybir.dt.float32r
Exp = mybir.ActivationFunctionType.Exp

B, S, HID = 4, 2048, 1024
H, DK = 16, 64
SK = 1280          # compacted+padded key count (keep ~ Binom(2048,.5), 11 sigma pad)
SKT = SK // 128    # 10 sk tiles
NPAIR = 4          # head pairs per core (8 heads = half the 16)

_PROG = None


def _build_program():
    nc = bacc.Bacc("TRN2", target_bir_lowering=False)

    xqt = nc.dram_tensor("xqt", [HID, S], fp32, kind="ExternalInput")
    xkvt = nc.dram_tensor("xkvt", [HID, SK], fp32, kind="ExternalInput")
    maskf = nc.dram_tensor("maskf", [128, SKT], fp32, kind="ExternalInput")
    wq = nc.dram_tensor("wq", [HID, 512], fp32, kind="ExternalInput")
    wk = nc.dram_tensor("wk", [HID, 512], fp32, kind="ExternalInput")
    wv = nc.dram_tensor("wv", [HID, 512], fp32, kind="ExternalInput")
    wo = nc.dram_tensor("wo", [512, HID], fp32, kind="ExternalInput")
    y = nc.dram_tensor("y", [S, HID], fp32, kind="ExternalOutput")

    # SBUF arena (fp32 word offsets per partition), resident + staging slab:
    #   KT   [0..5120)       K^T pair-major: KT[p, pair*1280 + sk]
    #   QT   [5120..13312)   Q^T: QT[p, pair*2048 + sq]
    #   YPN  [13312..21504)  normalized attn out^T: YPN[p, pair*2048 + sq]
    #   VP   [21504..31744)  pair*2560 + st*256 + [Va(64)|ma(64)|Vb(64)|mb(64)]
    #   SLAB [31744..41984)  input staging (XKVs / odd XQ quarters / WOs)
    arena = nc.alloc_sbuf_tensor("arena", [128, 41984], fp32)
    base = nc.lookup_mloc(arena).addr

    def at(name, words, off_words):
        return nc.alloc_sbuf_tensor_at(
            name, [128, words], fp32r, offset=base + off_words * 4
        )

    KT = at("KT", 5120, 0)
    QT = at("QT", 8192, 5120)
    YPN = at("YPN", 8192, 13312)
    VP = at("VP", 10240, 21504)
    # staging overlays
    WKs = at("WKs", 4096, 5120)       # QT region (phase A input)
    WVs = at("WVs", 4096, 9216)       # QT region (phase A input)
    XKVs = at("XKVs", 10240, 31744)   # slab, c-major: [:, c*1280 + sk]
    XQe = at("XQe", 4096, 13312)      # YPN region (even XQ quarters)
    WQs = at("WQs", 4096, 17408)      # YPN region (phase B input)
    XQo = at("XQo", 4096, 31744)      # slab (odd XQ quarters)
    WOs = at("WOs", 4096, 35840)      # slab (phase D input)

    with tile.TileContext(nc) as tc, ExitStack() as ctx:
        misc = ctx.enter_context(tc.tile_pool(name="misc", bufs=1))
        pt_pool = ctx.enter_context(tc.tile_pool(name="ptp", bufs=3))
        ev_pool = ctx.enter_context(tc.tile_pool(name="evp", bufs=2))
        rc_pool = ctx.enter_context(tc.tile_pool(name="rcp", bufs=2))
        ps_e = ctx.enter_context(tc.tile_pool(name="pse", bufs=3, space="PSUM"))
        ps_y = ctx.enter_context(tc.tile_pool(name="psy", bufs=2, space="PSUM"))

        maskt = misc.tile([128, SKT], fp32)
        nc.sync.dma_start(maskt[:], maskf[:])

        # DMA stream ordered by first use: WK, XKV lo-half, WV, XKV hi-half
        for c in range(8):
            nc.sync.dma_start(WKs[:, c * 512:(c + 1) * 512],
                              wk[c * 128:(c + 1) * 128, :].bitcast(fp32r))
        for c in range(8):
            nc.sync.dma_start(XKVs[:, c * SK: c * SK + 640],
                              xkvt[c * 128:(c + 1) * 128, 0:640].bitcast(fp32r))
        for c in range(8):
            nc.sync.dma_start(WVs[:, c * 512:(c + 1) * 512],
                              wv[c * 128:(c + 1) * 128, :].bitcast(fp32r))
        for c in range(8):
            nc.sync.dma_start(XKVs[:, c * SK + 640:(c + 1) * SK],
                              xkvt[c * 128:(c + 1) * 128, 640:SK].bitcast(fp32r))
        for c in range(8):
            nc.sync.dma_start(WQs[:, c * 512:(c + 1) * 512],
                              wq[c * 128:(c + 1) * 128, :].bitcast(fp32r))
        for c in range(8):
            nc.sync.dma_start(XQe[:, c * 512:(c + 1) * 512],
                              xqt[c * 128:(c + 1) * 128, 0:512].bitcast(fp32r))

        # init VP mask columns (denominator ones, masked) while DMAs stream
        ones = misc.tile([128, 64], fp32)
        nc.vector.memset(ones[:], 1.0)
        for p in range(NPAIR):
            for st in range(SKT):
                for hh in range(2):
                    o = p * 2560 + st * 256 + hh * 128 + 64
                    nc.vector.tensor_scalar_mul(
                        VP[:, o:o + 64], ones[:].bitcast(fp32r),
                        maskt[:, st:st + 1])

        # ---- Phase A: K^T -> KT, V (masked) -> VP (ordered by DMA arrival)
        def ktr_chunk(p, off, n):
            pk = ps_y.tile([128, 512], fp32, name="psyt")
            for c in range(8):
                nc.tensor.matmul(
                    pk[:, 0:n],
                    WKs[:, c * 512 + p * 128: c * 512 + (p + 1) * 128],
                    XKVs[:, c * SK + off: c * SK + off + n],
                    start=(c == 0), stop=(c == 7))
            nc.vector.tensor_copy(
                KT[:, p * SK + off: p * SK + off + n],
                pk[:, 0:n].bitcast(fp32r))

        def v_tile(st):
            pv = ps_y.tile([128, 512], fp32, name="psyt")
            for c in range(8):
                nc.tensor.matmul(
                    pv[:],
                    XKVs[:, c * SK + st * 128: c * SK + (st + 1) * 128],
                    WVs[:, c * 512:(c + 1) * 512],
                    start=(c == 0), stop=(c == 7))
            for h in range(8):
                o = (h // 2) * 2560 + st * 256 + (h % 2) * 128
                nc.vector.tensor_scalar_mul(
                    VP[:, o:o + 64], pv[:, h * 64:(h + 1) * 64].bitcast(fp32r),
                    maskt[:, st:st + 1])

        for p in range(NPAIR):
            ktr_chunk(p, 0, 512)
        for st in range(5):
            v_tile(st)
        for p in range(NPAIR):
            ktr_chunk(p, 512, 512)
        for p in range(NPAIR):
            ktr_chunk(p, 1024, 256)
        for st in range(5, SKT):
            v_tile(st)

        # ---- Phase B: Q^T -> QT (XQ streamed in quarters, 2 bufs) ----
        xqbufs = [XQe, XQo]
        for q in range(4):
            if q + 1 < 4:
                nb = xqbufs[(q + 1) % 2]
                for c in range(8):
                    nc.sync.dma_start(
                        nb[:, c * 512:(c + 1) * 512],
                        xqt[c * 128:(c + 1) * 128,
                            (q + 1) * 512:(q + 2) * 512].bitcast(fp32r))
            buf = xqbufs[q % 2]
            for p in range(NPAIR):
                pq = ps_y.tile([128, 512], fp32, name="psyt")
                for c in range(8):
                    nc.tensor.matmul(
                        pq[:],
                        WQs[:, c * 512 + p * 128: c * 512 + (p + 1) * 128],
                        buf[:, c * 512:(c + 1) * 512],
                        start=(c == 0), stop=(c == 7))
                nc.vector.tensor_copy(
                    QT[:, p * 2048 + q * 512: p * 2048 + (q + 1) * 512],
                    pq[:].bitcast(fp32r))

        # prefetch W_O during phase C
        for c in range(4):
            nc.sync.dma_start(WOs[:, c * 1024:(c + 1) * 1024],
                              wo[c * 128:(c + 1) * 128, :].bitcast(fp32r))

        # ---- Phase C + D interleaved ----
        def d_group(m, no):
            pd = ps_y.tile([128, 512], fp32, name="psyt")
            for tt in range(NPAIR):
                nc.tensor.matmul(
                    pd[:],
                    YPN[:, tt * 2048 + m * 128: tt * 2048 + (m + 1) * 128],
                    WOs[:, tt * 1024 + no * 512: tt * 1024 + no * 512 + 512],
                    start=(tt == 0), stop=(tt == 3))
            ob = ev_pool.tile([128, 512], fp32)
            nc.vector.tensor_copy(ob[:], pd[:])
            nc.sync.dma_start(
                y[m * 128:(m + 1) * 128, no * 512: no * 512 + 512], ob[:])

        # One D output-projection group (~853ns PE) is slotted after each
        # ACT-bound C combo (~1047ns/tile ACT vs ~853ns PE), one sq-block
        # behind so the YPN columns it reads are final.
        pending = []
        with nc.allow_low_precision(reason="fp32r is full-width fp32"):
            for n in range(4):
                combo_idx = 0
                for p in range(NPAIR):
                    for hh in range(2):
                        py = ps_y.tile([128, 512], fp32, name="psyt")
                        for sp in range(5):
                            pe = ps_e.tile([128, 1024], fp32)
                            for half in range(2):
                                st = sp * 2 + half
                                nc.tensor.matmul(
                                    pe[:, half * 512:(half + 1) * 512],
                                    KT[hh * 64:(hh + 1) * 64,
                                       p * SK + st * 128: p * SK + (st + 1) * 128],
                                    QT[hh * 64:(hh + 1) * 64,
                                       p * 2048 + n * 512: p * 2048 + n * 512 + 512],
                                    start=True, stop=True,
                                    tile_position=(hh * 64, 0))
                            pt = pt_pool.tile([128, 1024], fp32r)
                            nc.scalar.activation(pt[:], pe[:], Exp, scale=0.125)
                            for half in range(2):
                                st = sp * 2 + half
                                nc.tensor.matmul(
                                    py[:],
                                    VP[:, p * 2560 + st * 256 + hh * 128:
                                       p * 2560 + st * 256 + hh * 128 + 128],
                                    pt[:, half * 512:(half + 1) * 512],
                                    start=(sp == 0 and half == 0),
                                    stop=(sp == 4 and half == 1))
                        rc = rc_pool.tile([64, 512], fp32r)
                        nc.vector.reciprocal(rc[:], py[64:128, :].bitcast(fp32r))
                        nc.vector.tensor_mul(
                            YPN[hh * 64:(hh + 1) * 64,
                                p * 2048 + n * 512: p * 2048 + n * 512 + 512],
                            py[0:64, :].bitcast(fp32r), rc[:])
                        if pending:
                            d_group(*pending[combo_idx])
                        combo_idx += 1
                pending = [(m, no) for m in range(n * 4, n * 4 + 4)
                           for no in range(2)]
            for m, no in pending:
                d_group(m, no)

    nc.finalize()
    return nc


def _get_program():
    global _PROG
    if _PROG is None:
        _PROG = _build_program()
    return _PROG


def _make_in_maps(inputs):
    X_Q = np.asarray(inputs["X_Q"], dtype=np.float32)
    X_KV = np.asarray(inputs["X_KV"], dtype=np.float32)
    mask = np.asarray(inputs["key_padding_mask"])
    W_Q = np.asarray(inputs["W_Q"], dtype=np.float32)
    W_K = np.asarray(inputs["W_K"], dtype=np.float32)
    W_V = np.asarray(inputs["W_V"], dtype=np.float32)
    W_O = np.asarray(inputs["W_O"], dtype=np.float32)
    in_maps = []
    for core in range(8):
        b, half = core // 2, core % 2
        idx = np.flatnonzero(~mask[b].astype(bool))
        nk = len(idx)
        assert nk <= SK, f"kept keys {nk} exceed padded SK={SK}"
        xkvc = np.zeros((SK, HID), dtype=np.float32)
        xkvc[:nk] = X_KV[b][idx]
        maskv = (np.arange(SK) < nk).astype(np.float32)
        in_maps.append({
            "xqt": np.ascontiguousarray(X_Q[b].T),
            "xkvt": np.ascontiguousarray(xkvc.T),
            "maskf": np.ascontiguousarray(maskv.reshape(SKT, 128).T),
            "wq": np.ascontiguousarray(W_Q[:, half * 512:(half + 1) * 512]),
            "wk": np.ascontiguousarray(W_K[:, half * 512:(half + 1) * 512]),
            "wv": np.ascontiguousarray(W_V[:, half * 512:(half + 1) * 512]),
            "wo": np.ascontiguousarray(W_O[half * 512:(half + 1) * 512, :]),
        })
    return in_maps


def kernel(**inputs):
    nc = _get_program()
    in_maps = _make_in_maps(inputs)
    res = run_bass_kernel_spmd(nc, in_maps, core_ids=list(range(8)))
    out = np.empty((B, S, HID), dtype=np.float32)
    for b in range(B):
        out[b] = res.results[2 * b]["y"] + res.results[2 * b + 1]["y"]
    return out


# revision 20
# speedup vs baseline: 1.0381x; 1.0381x over previous
import sys

sys.path.insert(0, "/opt/trn_rl_repo")
import numpy as np
from contextlib import ExitStack

from concourse import bacc
import concourse.tile as tile
from concourse import mybir
from concourse.bass_utils import run_bass_kernel_spmd

fp32 = mybir.dt.float32
fp32r = mybir.dt.float32r
Exp = mybir.ActivationFunctionType.Exp

B, S, HID = 4, 2048, 1024
H, DK = 16, 64
SK = 1152          # compacted+padded key count (max kept keys = 1036, 5 sigma pad)
SKT = SK // 128    # 9 sk tiles
NPAIR = 4          # head pairs per core (8 heads = half the 16)

_PROG = None


def _build_program():
    nc = bacc.Bacc("TRN2", target_bir_lowering=False)

    xqt = nc.dram_tensor("xqt", [HID, S], fp32, kind="ExternalInput")
    xkvt = nc.dram_tensor("xkvt", [HID, SK], fp32, kind="ExternalInput")
    maskf = nc.dram_tensor("maskf", [128, SKT], fp32, kind="ExternalInput")
    wq = nc.dram_tensor("wq", [HID, 512], fp32, kind="ExternalInput")
    wk = nc.dram_tensor("wk", [HID, 512], fp32, kind="ExternalInput")
    wv = nc.dram_tensor("wv", [HID, 512], fp32, kind="ExternalInput")
    wo = nc.dram_tensor("wo", [512, HID], fp32, kind="ExternalInput")
    y = nc.dram_tensor("y", [S, HID], fp32, kind="ExternalOutput")

    # SBUF arena (fp32 word offsets per partition), resident + staging slab:
    #   KT   [0..4608)       K^T pair-major: KT[p, pair*1152 + sk]
    #   QT   [4608..12800)   Q^T: QT[p, pair*2048 + sq]
    #   YPN  [12800..20992)  normalized attn out^T: YPN[p, pair*2048 + sq]
    #   VP   [20992..30208)  pair*2304 + st*256 + [Va(64)|ma(64)|Vb(64)|mb(64)]
    #   SLAB [30208..39424)  input staging (XKVs / odd XQ quarters / WOs)
    #   WQS  [39424..43520)  W_Q staging (own bytes: no fence, early DMA)
    arena = nc.alloc_sbuf_tensor("arena", [128, 43520], fp32)
    base = nc.lookup_mloc(arena).addr

    def at(name, words, off_words):
        return nc.alloc_sbuf_tensor_at(
            name, [128, words], fp32r, offset=base + off_words * 4
        )

    KT = at("KT", 4608, 0)
    QT = at("QT", 8192, 4608)
    YPN = at("YPN", 8192, 12800)
    VP = at("VP", 9216, 20992)
    WQs = at("WQs", 4096, 39424)
    # staging overlays
    WKs = at("WKs", 4096, 4608)       # QT pairs 0-1 (phase A input)
    WVs = at("WVs", 4096, 8704)       # QT pairs 2-3 (phase A input)
    XKVs = at("XKVs", 9216, 30208)    # slab, c-major: [:, c*1152 + sk]
    XQe = at("XQe", 4096, 12800)      # YPN pairs 0-1 (even XQ quarters)
    XQo = at("XQo", 4096, 30208)      # slab cols 0..4096 (odd XQ quarters)
    WOs = at("WOs", 4096, 34304)      # slab cols 4096..8192 (phase D input)

    with tile.TileContext(nc) as tc, ExitStack() as ctx:
        misc = ctx.enter_context(tc.tile_pool(name="misc", bufs=1))
        pt_pool = ctx.enter_context(tc.tile_pool(name="ptp", bufs=3))
        ev_pool = ctx.enter_context(tc.tile_pool(name="evp", bufs=3))
        rc_pool = ctx.enter_context(tc.tile_pool(name="rcp", bufs=2))
        ps_e = ctx.enter_context(tc.tile_pool(name="pse", bufs=3, space="PSUM"))
        ps_y = ctx.enter_context(tc.tile_pool(name="psy", bufs=2, space="PSUM"))

        maskt = misc.tile([128, SKT], fp32)
        nc.sync.dma_start(maskt[:], maskf[:])

        # DMA stream ordered by first use: WK, XKV thirds, WV, WQ, XQ q0
        for c in range(8):
            nc.sync.dma_start(WKs[:, c * 512:(c + 1) * 512],
                              wk[c * 128:(c + 1) * 128, :].bitcast(fp32r))
        for lo, hi in ((0, 384), (384, 768)):
            for c in range(8):
                nc.sync.dma_start(
                    XKVs[:, c * SK + lo: c * SK + hi],
                    xkvt[c * 128:(c + 1) * 128, lo:hi].bitcast(fp32r))
        for c in range(8):
            nc.sync.dma_start(WVs[:, c * 512:(c + 1) * 512],
                              wv[c * 128:(c + 1) * 128, :].bitcast(fp32r))
        for c in range(8):
            nc.sync.dma_start(XKVs[:, c * SK + 768:(c + 1) * SK],
                              xkvt[c * 128:(c + 1) * 128, 768:SK].bitcast(fp32r))
        for c in range(8):
            nc.sync.dma_start(WQs[:, c * 512:(c + 1) * 512],
                              wq[c * 128:(c + 1) * 128, :].bitcast(fp32r))
        for c in range(8):
            nc.sync.dma_start(XQe[:, c * 512:(c + 1) * 512],
                              xqt[c * 128:(c + 1) * 128, 0:512].bitcast(fp32r))

        ones = misc.tile([128, 64], fp32)
        nc.vector.memset(ones[:], 1.0)

        # ---- Phase A: K^T -> KT, V (masked) -> VP (ordered by DMA arrival)
        def ktr_chunk(p, off):
            pk = ps_y.tile([128, 512], fp32, name="psyt")
            for c in range(8):
                nc.tensor.matmul(
                    pk[:, 0:384],
                    WKs[:, c * 512 + p * 128: c * 512 + (p + 1) * 128],
                    XKVs[:, c * SK + off: c * SK + off + 384],
                    start=(c == 0), stop=(c == 7))
            nc.vector.tensor_copy(
                KT[:, p * SK + off: p * SK + off + 384],
                pk[:, 0:384].bitcast(fp32r))

        def v_tile(st):
            # 3-buf ps_e pool (idle until phase C) so DVE mul lag never
            # blocks the psum rotation
            pv = ps_e.tile([128, 1024], fp32, name="pe")
            for c in range(8):
                nc.tensor.matmul(
                    pv[:, 0:512],
                    XKVs[:, c * SK + st * 128: c * SK + (st + 1) * 128],
                    WVs[:, c * 512:(c + 1) * 512],
                    start=(c == 0), stop=(c == 7))
            for h in range(8):
                o = (h // 2) * 2304 + st * 256 + (h % 2) * 128
                nc.vector.tensor_scalar_mul(
                    VP[:, o:o + 64], pv[:, h * 64:(h + 1) * 64].bitcast(fp32r),
                    maskt[:, st:st + 1])

        for p in range(NPAIR):
            ktr_chunk(p, 0)
        for p in range(NPAIR):
            ktr_chunk(p, 384)
        for st in range(6):
            v_tile(st)
        for p in range(NPAIR):
            ktr_chunk(p, 768)
        for st in range(6, SKT):
            v_tile(st)

        # q1 + W_O staged into slab bytes last read by phase A (auto-fenced)
        for c in range(8):
            nc.sync.dma_start(XQo[:, c * 512:(c + 1) * 512],
                              xqt[c * 128:(c + 1) * 128, 512:1024].bitcast(fp32r))
        for c in range(4):
            nc.sync.dma_start(WOs[:, c * 1024:(c + 1) * 1024],
                              wo[c * 128:(c + 1) * 128, :].bitcast(fp32r))

        # ---- Phase B seed: Q^T for query block q0, all pairs ----
        def b_group(q, p, buf):
            pq = ps_y.tile([128, 512], fp32, name="psyt")
            for c in range(8):
                nc.tensor.matmul(
                    pq[:],
                    WQs[:, c * 512 + p * 128: c * 512 + (p + 1) * 128],
                    buf[:, c * 512:(c + 1) * 512],
                    start=(c == 0), stop=(c == 7))
            nc.vector.tensor_copy(
                QT[:, p * 2048 + q * 512: p * 2048 + (q + 1) * 512],
                pq[:].bitcast(fp32r))

        for p in range(NPAIR):
            b_group(0, p, XQe)

        # q2 reuses XQe once the q0 block above consumed it
        for c in range(8):
            nc.sync.dma_start(XQe[:, c * 512:(c + 1) * 512],
                              xqt[c * 128:(c + 1) * 128, 1024:1536].bitcast(fp32r))

        # init VP mask columns (denominator ones, masked) now that DVE is
        # idle; stage-1 pairs (2,3) first so the first combos' A*V reads
        # are ready in time
        for p in (2, 3, 0, 1):
            for hh in range(2):
                for st in range(SKT):
                    o = p * 2304 + st * 256 + hh * 128 + 64
                    nc.vector.tensor_scalar_mul(
                        VP[:, o:o + 64], ones[:].bitcast(fp32r),
                        maskt[:, st:st + 1])

        # ---- Phases C (attention) and D (out-proj), B fillers inside C ----
        def d_group(m, no):
            pd = ps_y.tile([128, 512], fp32, name="psyt")
            for tt in range(NPAIR):
                nc.tensor.matmul(
                    pd[:],
                    YPN[:, tt * 2048 + m * 128: tt * 2048 + (m + 1) * 128],
                    WOs[:, tt * 1024 + no * 512: tt * 1024 + no * 512 + 512],
                    start=(tt == 0), stop=(tt == 3))
            ob = ev_pool.tile([128, 512], fp32)
            nc.vector.tensor_copy(ob[:], pd[:])
            nc.sync.dma_start(
                y[m * 128:(m + 1) * 128, no * 512: no * 512 + 512], ob[:])

        NST = (2, 2, 2, 2, 1)  # sk tiles per exp stage (9 total)

        def c_combo(n, p, hh, host=None):
            # software-pipelined: pe/exp run 3 stages ahead of the A*V
            # matmuls; hosted filler/out-proj work runs in the exp ramp so
            # PE never waits on ACT.  py is allocated AFTER host() so the
            # 2-buf psum rotation never recycles an open accumulator.
            qsl = QT[hh * 64:(hh + 1) * 64,
                     p * 2048 + n * 512: p * 2048 + n * 512 + 512]
            pts = []

            def emit_pe(k):
                pe = ps_e.tile([128, 1024], fp32)
                w = 512 * NST[k]
                for j in range(NST[k]):
                    st = 2 * k + j
                    nc.tensor.matmul(
                        pe[:, j * 512:(j + 1) * 512],
                        KT[hh * 64:(hh + 1) * 64,
                           p * SK + st * 128: p * SK + (st + 1) * 128],
                        qsl, start=True, stop=True,
                        tile_position=(hh * 64, 0))
                pt = pt_pool.tile([128, 1024], fp32r)
                nc.scalar.activation(pt[:, 0:w], pe[:, 0:w], Exp, scale=0.125)
                pts.append(pt)

            def emit_py(k, py):
                for j in range(NST[k]):
                    st = 2 * k + j
                    nc.tensor.matmul(
                        py[:],
                        VP[:, p * 2304 + st * 256 + hh * 128:
                           p * 2304 + st * 256 + hh * 128 + 128],
                        pts[k][:, j * 512:(j + 1) * 512],
                        start=(st == 0), stop=(st == SKT - 1))

            for k in range(3):
                emit_pe(k)
            if host is not None:
                host()
            py = ps_y.tile([128, 512], fp32, name="psyt")
            for k in range(3, 5):
                emit_py(k - 3, py)
                emit_pe(k)
            for k in range(2, 5):
                emit_py(k, py)
            rc = rc_pool.tile([64, 512], fp32r)
            nc.vector.reciprocal(rc[:], py[64:128, :].bitcast(fp32r))
            nc.vector.tensor_mul(
                YPN[hh * 64:(hh + 1) * 64,
                    p * 2048 + n * 512: p * 2048 + n * 512 + 512],
                py[0:64, :].bitcast(fp32r), rc[:])

        # B fillers for stage 1: one ~1.7us group behind each ~4.8us combo.
        # Each (q,p) filler precedes the first stage-1 combo reading QT(q,p);
        # q3's XQo DMA issues only after the last q1 reader (idx 5).
        fillers = [(1, 2), (1, 3), (2, 2), (2, 3), (1, 0), (1, 1),
                   (2, 0), (3, 2), (3, 3), (2, 1), (3, 0), (3, 1)]
        qbuf = {1: XQo, 2: XQe, 3: XQo}

        def filler_host(i):
            def host():
                fq, fp = fillers[i]
                b_group(fq, fp, qbuf[fq])
                if i == 5:
                    for c in range(8):
                        nc.sync.dma_start(
                            XQo[:, c * 512:(c + 1) * 512],
                            xqt[c * 128:(c + 1) * 128,
                                1536:2048].bitcast(fp32r))
            return host

        def d_host(groups):
            def host():
                for m, no in groups:
                    d_group(m, no)
            return host

        with nc.allow_low_precision(reason="fp32r is full-width fp32"):
            # Stage 1: head pairs 2-3 over all query blocks, B fillers inside
            idx = 0
            for n in range(4):
                for p in (2, 3):
                    for hh in range(2):
                        c_combo(n, p, hh,
                                filler_host(idx) if idx < len(fillers)
                                else None)
                        idx += 1

            # Stage 2: head pairs 0-1; D(n-1) groups ride in block n's shadow
            for n in range(4):
                dlist = ([(m, no) for m in range((n - 1) * 4, n * 4)
                          for no in range(2)] if n >= 1 else [])
                di = 0
                for p in (0, 1):
                    for hh in range(2):
                        c_combo(n, p, hh,
                                d_host(dlist[di:di + 2]) if di < len(dlist)
                                else None)
                        di += 2
            for m in range(12, 16):
                for no in range(2):
                    d_group(m, no)

    nc.finalize()
    return nc


def _get_program():
    global _PROG
    if _PROG is None:
        _PROG = _build_program()
    return _PROG


def _make_in_maps(inputs):
    X_Q = np.asarray(inputs["X_Q"], dtype=np.float32)
    X_KV = np.asarray(inputs["X_KV"], dtype=np.float32)
    mask = np.asarray(inputs["key_padding_mask"])
    W_Q = np.asarray(inputs["W_Q"], dtype=np.float32)
    W_K = np.asarray(inputs["W_K"], dtype=np.float32)
    W_V = np.asarray(inputs["W_V"], dtype=np.float32)
    W_O = np.asarray(inputs["W_O"], dtype=np.float32)
    in_maps = []
    for core in range(8):
        b, half = core // 2, core % 2
        idx = np.flatnonzero(~mask[b].astype(bool))
        nk = len(idx)
        assert nk <= SK, f"kept keys {nk} exceed padded SK={SK}"
        xkvc = np.zeros((SK, HID), dtype=np.float32)
        xkvc[:nk] = X_KV[b][idx]
        maskv = (np.arange(SK) < nk).astype(np.float32)
        in_maps.append({
            "xqt": np.ascontiguousarray(X_Q[b].T),
            "xkvt": np.ascontiguousarray(xkvc.T),
            "maskf": np.ascontiguousarray(maskv.reshape(SKT, 128).T),
            "wq": np.ascontiguousarray(W_Q[:, half * 512:(half + 1) * 512]),
            "wk": np.ascontiguousarray(W_K[:, half * 512:(half + 1) * 512]),
            "wv": np.ascontiguousarray(W_V[:, half * 512:(half + 1) * 512]),
            "wo": np.ascontiguousarray(W_O[half * 512:(half + 1) * 512, :]),
        })
    return in_maps


def kernel(**inputs):
    nc = _get_program()
    in_maps = _make_in_maps(inputs)
    res = run_bass_kernel_spmd(nc, in_maps, core_ids=list(range(8)))
    out = np.empty((B, S, HID), dtype=np.float32)
    for b in range(B):
        out[b] = res.results[2 * b]["y"] + res.results[2 * b + 1]["y"]
    return out
